# revision 16
# baseline (speedup 1.0000x reference)
"""Trainium2 Bass kernel for nn_KWinnersBoost (top-k masking with boosting).

Takes FULL inputs, returns FULL outputs. Row-parallel across 8 NeuronCores
(512 rows each), SPMD via run_bass_kernel_spmd.

Device computes the per-row top-164 mask of x = tensor (valid when
boost_tensor == 0, verified on host; ties/exotic/estimator-miss rows are
repaired on host row-by-row via the device count verification):

  Per 128-row chunk (chunk-major, pipelined across engines): 3 ACT
  sign-counting passes (pass 0 at t0=2.054 hidden under the input DMA, then
  a poly-ln interpolation and one damped per-row secant step targeting
  count ~143) land threshold t2 with need = 164 - count(x > t2) in [0, 40)
  for almost every row. The 165th-largest value is then extracted on DVE:
  zb = x * (x <= t2), top-8 of each of 16 512-wide blocks (InstMax), then a
  40-deep merge of the 128 block candidates with tiny [P,128] ops.
  LO = merged[need]; final mask out = sign(x - LO) as int8 on ACT with the
  row count accumulated for verification.

boost_out is reconstructed on host: bo = where(out, 0, relu(max(x))*bp) --
the global max is computed on host, so the device does no collectives.
"""

import os
import sys

if "/opt/trn_rl_repo" not in sys.path:
    sys.path.insert(0, "/opt/trn_rl_repo")

import numpy as np

import concourse.bacc as bacc
import concourse.tile as tile
from concourse import mybir
from concourse.bass_utils import run_bass_kernel_spmd

F32 = mybir.dt.float32
I8 = mybir.dt.int8
I32 = mybir.dt.int32

B, E = 4096, 8192
N_CORES = 8
ROWS = B // N_CORES          # 512
P = 128
NCH = ROWS // P              # 4 chunks
K = 164
H = E // 2

T0 = 2.054                   # pass-0 global threshold (2% tail of N(0,1))
LN_SLOPE = 0.39358           # 1/(t0 + 1/t0)
BETA_C = 2.4e-3              # model spacing near the 164th value
SLOPE_LO = 8e-4
SLOPE_HI = 7.2e-3
M_TARGET = 144.0             # ln-interp target; need = K - c2 lands in [0, 40)
NB = 16                      # endgame blocks per row
BW = E // NB                 # 512
DEPTH = 40                   # merged candidate depth

AluOp = mybir.AluOpType
Sign = mybir.ActivationFunctionType.Sign
AxX = mybir.AxisListType.X


def _build_body(tc, x_d, out_d, cnt_d, ctx):
    nc = tc.nc

    xpool = ctx.enter_context(tc.tile_pool(name="xpool", bufs=1))
    scrF = ctx.enter_context(tc.tile_pool(name="scrF", bufs=2))   # [P,E] slots
    jnk = ctx.enter_context(tc.tile_pool(name="jnk", bufs=1))     # ACT junk
    sm = ctx.enter_context(tc.tile_pool(name="sm", bufs=2))       # [P,128] f32
    st = ctx.enter_context(tc.tile_pool(name="st", bufs=1))

    x_t = [xpool.tile([P, E], F32, tag=f"x{c}", name=f"x{c}") for c in range(NCH)]

    def stt(tag, w=1):
        return st.tile([P, w], F32, tag=tag, name=tag)

    IOTA_I = st.tile([P, DEPTH], I32, tag="iotai", name="iotai")
    IOTA = st.tile([P, DEPTH], F32, tag="iota", name="iota")
    nc.gpsimd.iota(IOTA_I, [[1, DEPTH]], channel_multiplier=0)
    nc.vector.tensor_copy(IOTA, IOTA_I)

    CNT_OUT = st.tile([P, 3 * NCH], F32, tag="cntout", name="cntout")
    NT0 = st.tile([P, 1], F32, tag="nt0", name="nt0")
    nc.vector.memset(NT0, -float(T0))

    dma_engines = [nc.sync, nc.scalar, nc.gpsimd]
    RSPLIT = [(0, 43), (43, 86), (86, 128)]

    # input DMA: full-row chunks, rows 3-way split across queues so the
    # per-queue descriptor count (= partition count) is minimized
    for c in range(NCH):
        r0 = c * P
        for qi, (a, b) in enumerate(RSPLIT):
            dma_engines[qi].dma_start(
                out=x_t[c][a:b, :],
                in_=x_d[r0 + a : r0 + b, :],
            )

    def act_count(c, bias_ap, tag):
        """ACT full-row sign pass; returns count tile = (accum + E)/2."""
        ja = jnk.tile([P, E], I8, tag="jnk", name=f"ja_{tag}")
        sa = stt(f"sa_{tag}")
        nc.scalar.activation(
            out=ja, in_=x_t[c], func=Sign, bias=bias_ap, scale=1.0, accum_out=sa,
        )
        cn = stt(f"c_{tag}")
        nc.vector.tensor_scalar(
            out=cn, in0=sa, scalar1=float(E), scalar2=0.5,
            op0=AluOp.add, op1=AluOp.mult,
        )
        return cn

    for c in range(NCH):
        r0 = c * P
        # ---- pass1 @ t0 (hidden under load) --------------------------
        c1 = act_count(c, NT0, f"p1_{c}")

        # ---- ln-interp straight to the anchor target -----------------
        u = stt(f"u{c}")
        v = stt(f"v{c}")
        t2 = stt(f"t2_{c}")
        nc.vector.tensor_scalar(
            out=u, in0=c1, scalar1=float(1.0 / M_TARGET), scalar2=-1.0,
            op0=AluOp.mult, op1=AluOp.add,
        )
        nc.vector.tensor_scalar(
            out=v, in0=u, scalar1=float(-1.0 / 3.0), scalar2=0.5,
            op0=AluOp.mult, op1=AluOp.add,
        )
        nc.vector.tensor_tensor(out=v, in0=u, in1=v, op=AluOp.mult)
        nc.vector.tensor_scalar(
            out=v, in0=v, scalar1=-1.0, scalar2=1.0,
            op0=AluOp.mult, op1=AluOp.add,
        )
        nc.vector.tensor_tensor(out=v, in0=u, in1=v, op=AluOp.mult)
        nc.vector.tensor_scalar(
            out=t2, in0=v, scalar1=float(LN_SLOPE), scalar2=float(T0),
            op0=AluOp.mult, op1=AluOp.add,
        )
        nt2 = stt(f"nt2_{c}")
        nc.vector.tensor_scalar(
            out=nt2, in0=t2, scalar1=-1.0, scalar2=None, op0=AluOp.mult
        )

        # ---- pass2 @ t2 (exact anchor) ------------------------------
        c2 = act_count(c, nt2, f"p2_{c}")

        # ---- endgame: blockwise top-8 + 40-deep merge ---------------
        zb = scrF.tile([P, E], F32, tag="scrf", name=f"zb{c}")
        nc.vector.scalar_tensor_tensor(
            out=zb, in0=x_t[c], scalar=t2, in1=x_t[c],
            op0=AluOp.is_le, op1=AluOp.mult,
        )
        B128 = sm.tile([P, 8 * NB], F32, tag="sm", name=f"B128_{c}")
        for j in range(NB):
            nc.vector.max(B128[:, 8 * j : 8 * j + 8], zb[:, BW * j : BW * (j + 1)])
        B40 = st.tile([P, DEPTH], F32, tag=f"B40_{c}", name=f"B40_{c}")
        nc.vector.max(B40[:, 0:8], B128)
        cur = B128
        for rnd in range(1, DEPTH // 8):
            nxt = sm.tile([P, 8 * NB], F32, tag="sm", name=f"Bm{c}_{rnd}")
            nc.vector.scalar_tensor_tensor(
                out=nxt, in0=cur, scalar=B40[:, 8 * rnd - 1 : 8 * rnd], in1=cur,
                op0=AluOp.is_lt, op1=AluOp.mult,
            )
            nc.vector.max(B40[:, 8 * rnd : 8 * rnd + 8], nxt)
            cur = nxt

        # ---- selection: LO = B40[need], need = K - c2 ---------------
        need = stt(f"need{c}")
        nc.vector.tensor_scalar(
            out=need, in0=c2, scalar1=float(K), scalar2=-1.0,
            op0=AluOp.subtract, op1=AluOp.mult,
        )
        m40 = st.tile([P, DEPTH], F32, tag=f"m40_{c}", name=f"m40_{c}")
        nc.vector.tensor_scalar(
            out=m40, in0=IOTA, scalar1=need, scalar2=None, op0=AluOp.is_equal
        )
        nc.vector.tensor_tensor(out=m40, in0=m40, in1=B40, op=AluOp.mult)
        lo_t = stt(f"lo{c}")
        nc.vector.reduce_sum(out=lo_t, in_=m40, axis=AxX)
        nc.vector.tensor_scalar(
            out=lo_t, in0=lo_t, scalar1=0.0, scalar2=None, op0=AluOp.max
        )
        nlo = stt(f"nlo{c}")
        nc.vector.tensor_scalar(
            out=nlo, in0=lo_t, scalar1=-1.0, scalar2=None, op0=AluOp.mult
        )

        # ---- final: out = sign(x - LO) as i8 on ACT -----------------
        ot = scrF.tile([P, E], I8, tag="scrf", name=f"out{c}")
        nc.scalar.activation(
            out=ot, in_=x_t[c], func=Sign, bias=nlo, scale=1.0,
            accum_out=CNT_OUT[:, 3 * c : 3 * c + 1],
        )
        nc.vector.tensor_copy(CNT_OUT[:, 3 * c + 1 : 3 * c + 2], c2)
        nc.vector.tensor_copy(CNT_OUT[:, 3 * c + 2 : 3 * c + 3], lo_t)
        for qi, (a, b) in enumerate(RSPLIT):
            dma_engines[(qi + c) % 3].dma_start(
                out=out_d[r0 + a : r0 + b, :], in_=ot[a:b, :]
            )

    nc.gpsimd.dma_start(out=cnt_d[:, :], in_=CNT_OUT)


_NC_CACHE = None


def _build():
    global _NC_CACHE
    if _NC_CACHE is not None:
        return _NC_CACHE
    nc = bacc.Bacc(
        "TRN2", target_bir_lowering=False, debug=False, num_devices=N_CORES
    )
    x_d = nc.dram_tensor("tensor", [ROWS, E], F32, kind="ExternalInput").ap()
    out_d = nc.dram_tensor("out", [ROWS, E], I8, kind="ExternalOutput").ap()
    cnt_d = nc.dram_tensor("cnt", [P, 3 * NCH], F32, kind="ExternalOutput").ap()
    from contextlib import ExitStack

    with tile.TileContext(nc) as tc, ExitStack() as ctx:
        _build_body(tc, x_d, out_d, cnt_d, ctx)
    nc.compile()
    _NC_CACHE = nc
    return nc


_LAST_RESULTS = None
_LAST_NBAD = None


def _topk_row_mask(xr):
    """Exact reference-equivalent top-K mask for one row (boost == const)."""
    boosted = np.where(xr > 0, xr, np.float32(0))
    kth = np.partition(boosted, E - K)[E - K]
    mask = boosted > kth
    need = K - mask.sum()
    tie = (boosted == kth) & ~mask
    idx = np.nonzero(tie)[0][:need]
    mask[idx] = True
    return mask


def kernel(tensor, boost_tensor, boost_percent):
    global _LAST_RESULTS, _LAST_NBAD
    tensor = np.ascontiguousarray(np.asarray(tensor, dtype=np.float32))
    boost_tensor = np.asarray(boost_tensor, dtype=np.float32)
    bp = np.float32(np.asarray(boost_percent, dtype=np.float32).reshape(-1)[0])

    if boost_tensor.any():
        return _host_reference(tensor, boost_tensor, float(bp))

    nc = _build()
    in_maps = []
    for c in range(N_CORES):
        sl = slice(c * ROWS, (c + 1) * ROWS)
        in_maps.append({"tensor": tensor[sl]})
    trace = bool(int(os.environ.get("KW_TRACE", "0")))
    res = run_bass_kernel_spmd(
        nc, in_maps, core_ids=list(range(N_CORES)), trace=trace
    )
    _LAST_RESULTS = res

    out_i8 = np.concatenate([r["out"] for r in res.results], axis=0)
    mask = out_i8 > 0

    # verification: (signsum + E)/2 == K + 0.5 (exactly one x == LO in row)
    bad_rows = []
    for ci, r in enumerate(res.results):
        cnt = r["cnt"]  # [128, 12]
        for ch in range(NCH):
            tot = (cnt[:, 3 * ch] + float(E)) * 0.5
            bad = np.nonzero(tot != float(K) + 0.5)[0]
            for rr in bad:
                bad_rows.append(ci * ROWS + ch * P + int(rr))
    _LAST_NBAD = len(bad_rows)
    if len(bad_rows) > B // 4:
        return _host_reference(tensor, boost_tensor, float(bp))
    outm = mask if not bad_rows else mask.copy()
    for gr in bad_rows:
        m = _topk_row_mask(tensor[gr])
        mask[gr] = m                        # boost mask
        outm[gr] = m & (tensor[gr] > 0)     # activation mask

    out = outm.astype(np.float32)
    c_boost = np.float32(max(np.float32(0.0), tensor.max()) * bp)
    bo = np.where(mask, np.float32(0.0), c_boost).astype(np.float32)
    if out.sum() == 0:  # degenerate global case: defer to exact host path
        return _host_reference(tensor, boost_tensor, float(bp))
    return out, bo


def _host_reference(tensor, boost_tensor, bp):
    x = tensor.astype(np.float32)
    b = np.broadcast_to(boost_tensor.astype(np.float32), x.shape)
    max_val = max(0.0, float(x.max()))
    boost = (b + np.float32(max_val * bp)).astype(np.float32)
    boosted = (np.where(x > 0, x, np.float32(0)) + boost).astype(np.float32)
    kth = np.partition(boosted, E - K, axis=1)[:, E - K]
    mask = boosted > kth[:, None]
    need = K - mask.sum(1)
    tie = (boosted == kth[:, None]) & ~mask
    csum = np.cumsum(tie, axis=1)
    mask |= tie & (csum <= need[:, None])
    out = (mask & (x > 0)).astype(np.float32)
    if out.sum() == 0:
        out = mask.astype(np.float32)
    bo = np.where(mask, np.float32(0), boost).astype(np.float32)
    return out, bo


# revision 17
# speedup vs baseline: 4.0489x; 4.0489x over previous
"""Trainium2 Bass kernel for nn_KWinnersBoost (top-k masking with boosting).

Takes FULL inputs, returns FULL outputs. Row-parallel across 8 NeuronCores
(512 rows each), SPMD via run_bass_kernel_spmd.

Device computes the per-row top-164 mask of x = tensor (valid when
boost_tensor == 0, verified on host; ties/exotic/estimator-miss rows are
repaired on host row-by-row via the device count verification):

  Per 128-row chunk (chunk-major, pipelined across engines): 3 ACT
  sign-counting passes (pass 0 at t0=2.054 hidden under the input DMA, then
  a poly-ln interpolation and one damped per-row secant step targeting
  count ~143) land threshold t2 with need = 164 - count(x > t2) in [0, 40)
  for almost every row. The 165th-largest value is then extracted on DVE:
  zb = x * (x <= t2), top-8 of each of 16 512-wide blocks (InstMax), then a
  40-deep merge of the 128 block candidates with tiny [P,128] ops.
  LO = merged[need]; final mask out = sign(x - LO) as int8 on ACT with the
  row count accumulated for verification.

boost_out is reconstructed on host: bo = where(out, 0, relu(max(x))*bp) --
the global max is computed on host, so the device does no collectives.
"""

import os
import sys

if "/opt/trn_rl_repo" not in sys.path:
    sys.path.insert(0, "/opt/trn_rl_repo")

import numpy as np

import concourse.bacc as bacc
import concourse.tile as tile
from concourse import mybir
from concourse.bass_utils import run_bass_kernel_spmd

F32 = mybir.dt.float32
I8 = mybir.dt.int8
I32 = mybir.dt.int32

B, E = 4096, 8192
N_CORES = 8
ROWS = B // N_CORES          # 512
P = 128
NCH = ROWS // P              # 4 chunks
K = 164
H = E // 2

T0 = 2.054                   # pass-0 global threshold (2% tail of N(0,1))
LN_SLOPE = 0.39358           # 1/(t0 + 1/t0)
BETA_C = 2.4e-3              # model spacing near the 164th value
SLOPE_LO = 8e-4
SLOPE_HI = 7.2e-3
M_TARGET = 144.0             # ln-interp target; need = K - c2 lands in [0, 40)
NB = 16                      # endgame blocks per row
BW = E // NB                 # 512
DEPTH = 40                   # merged candidate depth

AluOp = mybir.AluOpType
Sign = mybir.ActivationFunctionType.Sign
AxX = mybir.AxisListType.X


def _build_body(tc, x_d, out_d, cnt_d, ctx):
    nc = tc.nc

    xpool = ctx.enter_context(tc.tile_pool(name="xpool", bufs=1))
    scrF = ctx.enter_context(tc.tile_pool(name="scrF", bufs=2))   # [P,E] slots
    jnk = ctx.enter_context(tc.tile_pool(name="jnk", bufs=1))     # ACT junk
    sm = ctx.enter_context(tc.tile_pool(name="sm", bufs=2))       # [P,128] f32
    st = ctx.enter_context(tc.tile_pool(name="st", bufs=1))

    x_t = [xpool.tile([P, E], F32, tag=f"x{c}", name=f"x{c}") for c in range(NCH)]

    def stt(tag, w=1):
        return st.tile([P, w], F32, tag=tag, name=tag)

    IOTA_I = st.tile([P, DEPTH], I32, tag="iotai", name="iotai")
    IOTA = st.tile([P, DEPTH], F32, tag="iota", name="iota")
    nc.gpsimd.iota(IOTA_I, [[1, DEPTH]], channel_multiplier=0)
    nc.vector.tensor_copy(IOTA, IOTA_I)

    CNT_OUT = st.tile([P, 3 * NCH], F32, tag="cntout", name="cntout")
    NT0 = st.tile([P, 1], F32, tag="nt0", name="nt0")
    nc.vector.memset(NT0, -float(T0))

    dma_engines = [nc.sync, nc.scalar]

    # input DMA: one full-row DMA per chunk (32KB DRAM lines -> fewest
    # descriptors), chunks alternating between the two HWDGE queues
    for c in range(NCH):
        r0 = c * P
        dma_engines[c % 2].dma_start(out=x_t[c], in_=x_d[r0 : r0 + P, :])

    def act_count(c, bias_ap, tag):
        """ACT full-row sign pass; returns count tile = (accum + E)/2."""
        ja = jnk.tile([P, E], I8, tag="jnk", name=f"ja_{tag}")
        sa = stt(f"sa_{tag}")
        nc.scalar.activation(
            out=ja, in_=x_t[c], func=Sign, bias=bias_ap, scale=1.0, accum_out=sa,
        )
        cn = stt(f"c_{tag}")
        nc.vector.tensor_scalar(
            out=cn, in0=sa, scalar1=float(E), scalar2=0.5,
            op0=AluOp.add, op1=AluOp.mult,
        )
        return cn

    for c in range(NCH):
        r0 = c * P
        # ---- pass1 @ t0 (hidden under load) --------------------------
        c1 = act_count(c, NT0, f"p1_{c}")

        # ---- ln-interp straight to the anchor target -----------------
        u = stt(f"u{c}")
        v = stt(f"v{c}")
        t2 = stt(f"t2_{c}")
        nc.vector.tensor_scalar(
            out=u, in0=c1, scalar1=float(1.0 / M_TARGET), scalar2=-1.0,
            op0=AluOp.mult, op1=AluOp.add,
        )
        nc.vector.tensor_scalar(
            out=v, in0=u, scalar1=float(-1.0 / 3.0), scalar2=0.5,
            op0=AluOp.mult, op1=AluOp.add,
        )
        nc.vector.tensor_tensor(out=v, in0=u, in1=v, op=AluOp.mult)
        nc.vector.tensor_scalar(
            out=v, in0=v, scalar1=-1.0, scalar2=1.0,
            op0=AluOp.mult, op1=AluOp.add,
        )
        nc.vector.tensor_tensor(out=v, in0=u, in1=v, op=AluOp.mult)
        nc.vector.tensor_scalar(
            out=t2, in0=v, scalar1=float(LN_SLOPE), scalar2=float(T0),
            op0=AluOp.mult, op1=AluOp.add,
        )
        nt2 = stt(f"nt2_{c}")
        nc.vector.tensor_scalar(
            out=nt2, in0=t2, scalar1=-1.0, scalar2=None, op0=AluOp.mult
        )

        # ---- pass2 @ t2 (exact anchor) ------------------------------
        c2 = act_count(c, nt2, f"p2_{c}")

        # ---- endgame: blockwise top-8 + 40-deep merge ---------------
        zb = scrF.tile([P, E], F32, tag="scrf", name=f"zb{c}")
        nc.vector.scalar_tensor_tensor(
            out=zb, in0=x_t[c], scalar=t2, in1=x_t[c],
            op0=AluOp.is_le, op1=AluOp.mult,
        )
        B128 = sm.tile([P, 8 * NB], F32, tag="sm", name=f"B128_{c}")
        for j in range(NB):
            nc.vector.max(B128[:, 8 * j : 8 * j + 8], zb[:, BW * j : BW * (j + 1)])
        B40 = st.tile([P, DEPTH], F32, tag=f"B40_{c}", name=f"B40_{c}")
        nc.vector.max(B40[:, 0:8], B128)
        cur = B128
        for rnd in range(1, DEPTH // 8):
            nxt = sm.tile([P, 8 * NB], F32, tag="sm", name=f"Bm{c}_{rnd}")
            nc.vector.scalar_tensor_tensor(
                out=nxt, in0=cur, scalar=B40[:, 8 * rnd - 1 : 8 * rnd], in1=cur,
                op0=AluOp.is_lt, op1=AluOp.mult,
            )
            nc.vector.max(B40[:, 8 * rnd : 8 * rnd + 8], nxt)
            cur = nxt

        # ---- selection: LO = B40[need], need = K - c2 ---------------
        need = stt(f"need{c}")
        nc.vector.tensor_scalar(
            out=need, in0=c2, scalar1=float(K), scalar2=-1.0,
            op0=AluOp.subtract, op1=AluOp.mult,
        )
        m40 = st.tile([P, DEPTH], F32, tag=f"m40_{c}", name=f"m40_{c}")
        nc.vector.tensor_scalar(
            out=m40, in0=IOTA, scalar1=need, scalar2=None, op0=AluOp.is_equal
        )
        nc.vector.tensor_tensor(out=m40, in0=m40, in1=B40, op=AluOp.mult)
        lo_t = stt(f"lo{c}")
        nc.vector.reduce_sum(out=lo_t, in_=m40, axis=AxX)
        nc.vector.tensor_scalar(
            out=lo_t, in0=lo_t, scalar1=0.0, scalar2=None, op0=AluOp.max
        )
        nlo = stt(f"nlo{c}")
        nc.vector.tensor_scalar(
            out=nlo, in0=lo_t, scalar1=-1.0, scalar2=None, op0=AluOp.mult
        )

        # ---- final: out = sign(x - LO) as i8 on ACT -----------------
        ot = scrF.tile([P, E], I8, tag="scrf", name=f"out{c}")
        nc.scalar.activation(
            out=ot, in_=x_t[c], func=Sign, bias=nlo, scale=1.0,
            accum_out=CNT_OUT[:, 3 * c : 3 * c + 1],
        )
        nc.vector.tensor_copy(CNT_OUT[:, 3 * c + 1 : 3 * c + 2], c2)
        nc.vector.tensor_copy(CNT_OUT[:, 3 * c + 2 : 3 * c + 3], lo_t)
        dma_engines[c % 2].dma_start(
            out=out_d[r0 : r0 + 64, :], in_=ot[0:64, :]
        )
        dma_engines[(c + 1) % 2].dma_start(
            out=out_d[r0 + 64 : r0 + P, :], in_=ot[64:P, :]
        )

    nc.sync.dma_start(out=cnt_d[:, :], in_=CNT_OUT)


_NC_CACHE = None


def _build():
    global _NC_CACHE
    if _NC_CACHE is not None:
        return _NC_CACHE
    nc = bacc.Bacc(
        "TRN2", target_bir_lowering=False, debug=False, num_devices=N_CORES
    )
    x_d = nc.dram_tensor("tensor", [ROWS, E], F32, kind="ExternalInput").ap()
    out_d = nc.dram_tensor("out", [ROWS, E], I8, kind="ExternalOutput").ap()
    cnt_d = nc.dram_tensor("cnt", [P, 3 * NCH], F32, kind="ExternalOutput").ap()
    from contextlib import ExitStack

    with tile.TileContext(nc) as tc, ExitStack() as ctx:
        _build_body(tc, x_d, out_d, cnt_d, ctx)
    nc.compile()
    _NC_CACHE = nc
    return nc


_LAST_RESULTS = None
_LAST_NBAD = None


def _topk_row_mask(xr):
    """Exact reference-equivalent top-K mask for one row (boost == const)."""
    boosted = np.where(xr > 0, xr, np.float32(0))
    kth = np.partition(boosted, E - K)[E - K]
    mask = boosted > kth
    need = K - mask.sum()
    tie = (boosted == kth) & ~mask
    idx = np.nonzero(tie)[0][:need]
    mask[idx] = True
    return mask


def kernel(tensor, boost_tensor, boost_percent):
    global _LAST_RESULTS, _LAST_NBAD
    tensor = np.ascontiguousarray(np.asarray(tensor, dtype=np.float32))
    boost_tensor = np.asarray(boost_tensor, dtype=np.float32)
    bp = np.float32(np.asarray(boost_percent, dtype=np.float32).reshape(-1)[0])

    if boost_tensor.any():
        return _host_reference(tensor, boost_tensor, float(bp))

    nc = _build()
    in_maps = []
    for c in range(N_CORES):
        sl = slice(c * ROWS, (c + 1) * ROWS)
        in_maps.append({"tensor": tensor[sl]})
    trace = bool(int(os.environ.get("KW_TRACE", "0")))
    res = run_bass_kernel_spmd(
        nc, in_maps, core_ids=list(range(N_CORES)), trace=trace
    )
    _LAST_RESULTS = res

    out_i8 = np.concatenate([r["out"] for r in res.results], axis=0)
    mask = out_i8 > 0

    # verification: (signsum + E)/2 == K + 0.5 (exactly one x == LO in row)
    bad_rows = []
    for ci, r in enumerate(res.results):
        cnt = r["cnt"]  # [128, 12]
        for ch in range(NCH):
            tot = (cnt[:, 3 * ch] + float(E)) * 0.5
            bad = np.nonzero(tot != float(K) + 0.5)[0]
            for rr in bad:
                bad_rows.append(ci * ROWS + ch * P + int(rr))
    _LAST_NBAD = len(bad_rows)
    if len(bad_rows) > B // 4:
        return _host_reference(tensor, boost_tensor, float(bp))
    outm = mask if not bad_rows else mask.copy()
    for gr in bad_rows:
        m = _topk_row_mask(tensor[gr])
        mask[gr] = m                        # boost mask
        outm[gr] = m & (tensor[gr] > 0)     # activation mask

    out = outm.astype(np.float32)
    c_boost = np.float32(max(np.float32(0.0), tensor.max()) * bp)
    bo = np.where(mask, np.float32(0.0), c_boost).astype(np.float32)
    if out.sum() == 0:  # degenerate global case: defer to exact host path
        return _host_reference(tensor, boost_tensor, float(bp))
    return out, bo


def _host_reference(tensor, boost_tensor, bp):
    x = tensor.astype(np.float32)
    b = np.broadcast_to(boost_tensor.astype(np.float32), x.shape)
    max_val = max(0.0, float(x.max()))
    boost = (b + np.float32(max_val * bp)).astype(np.float32)
    boosted = (np.where(x > 0, x, np.float32(0)) + boost).astype(np.float32)
    kth = np.partition(boosted, E - K, axis=1)[:, E - K]
    mask = boosted > kth[:, None]
    need = K - mask.sum(1)
    tie = (boosted == kth[:, None]) & ~mask
    csum = np.cumsum(tie, axis=1)
    mask |= tie & (csum <= need[:, None])
    out = (mask & (x > 0)).astype(np.float32)
    if out.sum() == 0:
        out = mask.astype(np.float32)
    bo = np.where(mask, np.float32(0), boost).astype(np.float32)
    return out, bo
